# revision 1
# baseline (speedup 1.0000x reference)
"""ConvSP (SPH smoothing-kernel convolution) Trainium2 Bass kernel.

Math (per batch b):
  out[o,i] = bias[o] + sum_k sum_j A_k[o,j] * relu(r^2 - |x_i - x_j + off_k|^2)^3
  A_k = knorm * weight[:,:,k] @ (data * coef),  coef = 1/(invmass*density)

Device strategy (8 cores = 2 batches x 4 i-blocks of 512):
  t_k[j,i] = r2 - |x_i - x_j + off_k|^2 is a rank-4 bilinear form:
      V_k[:,j] = [2x_j, 2y_j, 1, -|x_j|^2 + 2 x_j.off_k]          (lhsT, K=4)
      U_k[:,i] = [x_i, y_i, r2 - |off_k|^2 - |x_i|^2 - 2 x_i.off_k, 1]
  so each [128j x 512i] tile of t is ONE fp32 matmul. Then a single custom
  DVE op computes w = relu(t)^3 (PSUM fp32 -> SBUF bf16), and a bf16 matmul
  accumulates out[o,i] += A_kT[j,o].T @ w[j,i] into PSUM over all (k, j-chunk).
"""

import os
import re
import sys
import time

import numpy as np

for _p in ("/opt/trn_rl_repo", "/root/.axon_site/_ro/trn_rl_repo"):
    if os.path.isdir(_p) and _p not in sys.path:
        sys.path.append(_p)

import ml_dtypes  # noqa: E402

import concourse.bass as bass  # noqa: E402
import concourse.mybir as mybir  # noqa: E402
import concourse.tile as tile  # noqa: E402
from concourse.bass_utils import run_bass_kernel_spmd  # noqa: E402

# ---------------------------------------------------------------- constants
NDIM = 2
KSIZE = (3, 3)
DILATION = (0.05, 0.05)
RADIUS = 0.1
C_IN = 64
C_OUT = 64
B = 2
N = 2048
NCELLS = 9
R2 = RADIUS * RADIUS
KNORM = 315.0 / (64.0 * np.pi * RADIUS**9)

NCORES = 8
IBLK = 512          # i-columns per core
CHUNKS = N // 128   # 16 j-chunks of 128

F32 = mybir.dt.float32
BF16 = mybir.dt.bfloat16

_cache: dict = {}


# ------------------------------------------------- TileContext drain patch
# The walrus in this container rejects the Tile tail-drain when it carries
# more than ~2 sem waits ("Too many sync wait commands"). Split the waits
# over extra sync-engine NOPs, one wait each.
def _patch_tile_drain():
    if getattr(tile.TileContext, "_drain_patched", False):
        return
    import bass_rust
    from concourse.vector_clock import ScopedClock

    def _drain_and_barrier(self, tick_clock, wait_clock):
        drain_inst = self.nc.sync.drain()
        wait_clock.add_sem_waits(
            drain_inst.ins, ScopedClock({None: tick_clock.global_clock})
        )
        si = drain_inst.ins.sync_info
        waits = list(si.on_wait) if si is not None else []
        if len(waits) > 1:
            si.on_wait = waits[:1]
            drain_inst.ins.sync_info = si
            for w in waits[1:]:
                n = self.nc.sync.nop(nofuse=True, hint="drain_wait_split")
                n.ins.sync_info = bass_rust.SyncInfo(on_wait=[w], on_update=[])
        self.nc.all_engine_barrier()
        popped = self.nc._tile_sem_poison_stack.pop()
        assert popped is self._sem_poison
        self.nc.clear_and_free_semaphores(list(self.sems.allocated().values()))
        self.nc.all_engine_barrier()

    tile.TileContext._drain_and_barrier = _drain_and_barrier
    tile.TileContext._drain_patched = True


# --------------------------------------------- sync-wait legalization pass
# This walrus rejects instructions carrying more than ~1-2 sem waits. After
# Tile scheduling, move excess waits onto same-engine NoOps inserted right
# before the over-subscribed instruction (engines execute their stream in
# order, so semantics are identical).
_WAIT_LIMIT = 1


def _split_sync_waits(nc, limit=_WAIT_LIMIT):
    cnt = 0
    for f in nc.m.functions:
        for bb in f.blocks:
            changed = False
            out = []
            for inst in bb.instructions:
                si = inst.sync_info
                waits = list(si.on_wait) if si is not None else []
                if len(waits) > limit:
                    keep = waits[-limit:]
                    excess = waits[:-limit]
                    for j in range(0, len(excess), limit):
                        n = mybir.InstNoOp(
                            name=f"waitsplit_{cnt}",
                            engine=inst.engine,
                            ins=[],
                            outs=[],
                            sync_info=mybir.SyncInfo(
                                on_wait=excess[j : j + limit], on_update=[]
                            ),
                        )
                        cnt += 1
                        nc.register_instruction(n, overwrite=True)
                        out.append(n)
                    si.on_wait = keep
                    inst.sync_info = si
                    changed = True
                out.append(inst)
            if changed:
                bb.instructions = out
    return cnt


# ------------------------------------------------- custom DVE op: relu(x)^3
def _get_relu_cube():
    """Register (once) and return the RELU_CUBE custom DVE op, or None."""
    if os.environ.get("KERNEL_NO_CUSTOM_DVE"):
        return None
    if "relu_cube" in _cache:
        return _cache["relu_cube"]
    try:
        import concourse.dve_ops as dve_ops
        from concourse.dve_ops import DveOp
        from concourse.dve_spec import Spec, Src0, relu, sq

        name = "RELU_CUBE_ANT"
        r = relu(Src0)
        spec = Spec(
            body=sq(r) * r,
            reference=lambda in0, in1, s0, s1, imm2: (
                np.maximum(in0, 0.0) ** 3
            ).astype(np.float32),
        )
        if name not in dve_ops._SUB_OPCODE_FOR_NAME:
            placeholder = DveOp(name, spec, subdim=False, uops_sha={})
            dve_ops.OPS.append(placeholder)
            dve_ops._SUB_OPCODE_FOR_NAME[name] = (
                dve_ops._CUSTOM_DVE_ROW_BASE + len(dve_ops.OPS) - 1
            )
            assert dve_ops._SUB_OPCODE_FOR_NAME[name] < 0x20
            dve_ops.CUSTOM_DVE_SPECS[name] = spec
        # pin the uops sha by compiling once and parsing the mismatch error
        shas = {}
        for ver in ("v3", "v4"):
            try:
                dve_ops.OPS[-1].compile(ver)
            except ValueError as e:
                m = re.search(r"\b([0-9a-f]{8,})\b\s*≠", str(e))
                if m:
                    shas[ver] = m.group(1)
            except Exception:
                pass
        op = DveOp(name, spec, subdim=False, uops_sha=shas)
        dve_ops.OPS[-1] = op
        dve_ops.CUSTOM_DVE_SPECS[name] = spec
        # verify it now compiles clean for v3 (trn2)
        op.compile("v3")
        _cache["relu_cube"] = op
    except Exception as e:  # pragma: no cover - fallback path
        sys.stderr.write(f"[kernel] custom DVE unavailable ({e}); fallback\n")
        _cache["relu_cube"] = None
    return _cache["relu_cube"]


# ------------------------------------------------------------- device build
# Sharding: core = (b, i-block of 512 kd-sorted particles). Block-sparse:
# the host kd-sorts particles, tests (k, j-chunk) tiles against the core's
# i-block bounding box, and ships only surviving tiles as "work items" in
# item-indexed V/U/AT arrays (padded with zeros up to a common M).
IW = 512             # i-columns per core


def _build_nc(m_items):
    _patch_tile_drain()

    ngrp = m_items // 4
    nc = bass.Bass()
    # V/U packed over 4 row-groups: item m lives at partitions 32*(m%4)..+3,
    # column block m//4. Enables 4-way concurrent row-tiled t-matmuls.
    at_d = nc.declare_dram_parameter("AT", [128, m_items * C_OUT], BF16,
                                     isOutput=False)
    v_d = nc.declare_dram_parameter("V", [128, ngrp * 128], F32,
                                    isOutput=False)
    u_d = nc.declare_dram_parameter("U", [128, ngrp * IW], F32, isOutput=False)
    out_d = nc.declare_dram_parameter("out", [C_OUT, IW], F32, isOutput=True)

    from contextlib import ExitStack

    with tile.TileContext(nc) as tc, ExitStack() as ctx:
        const = ctx.enter_context(tc.tile_pool(name="const", bufs=1))
        wpool = ctx.enter_context(tc.tile_pool(name="w", bufs=4))
        qpool = ctx.enter_context(tc.tile_pool(name="q", bufs=4))
        tpool = ctx.enter_context(tc.tile_pool(name="t", bufs=3, space="PSUM"))
        opool = ctx.enter_context(tc.tile_pool(name="o", bufs=1, space="PSUM"))

        v_t = const.tile([128, ngrp * 128], F32)
        nc.sync.dma_start(v_t[:], v_d[:])
        u_t = const.tile([128, ngrp * IW], F32)
        nchunk = 6
        ug = -(-ngrp // nchunk)
        for qq in range(nchunk):
            lo, hi = qq * ug, min((qq + 1) * ug, ngrp)
            if lo >= hi:
                break
            nc.sync.dma_start(u_t[:, lo * IW : hi * IW],
                              u_d[:, lo * IW : hi * IW])
        at_t = const.tile([128, m_items * C_OUT], BF16)
        qsz = m_items * C_OUT // 4
        for qq in range(4):
            nc.sync.dma_start(at_t[:, qq * qsz : (qq + 1) * qsz],
                              at_d[:, qq * qsz : (qq + 1) * qsz])

        # main-matmul accumulator: even items -> partitions 0:64,
        # odd items -> 64:128 (2-way col-tiled concurrency); added at the end
        out_ps = opool.tile([128, IW], F32)

        # software pipeline: t-matmuls for group g run while DVE/ACT cube
        # group g-1 and PE then accumulates g-1's mains. Items are paired
        # into [128, 2*IW] PSUM tiles so each ACT/DVE op covers two items.
        pend = []  # (item_idx, w_tile, half) awaiting the main matmul
        for g in range(ngrp):
            for half in range(2):
                t_ps = tpool.tile([128, 2 * IW], F32)
                for r in range(2):
                    m = g * 4 + half * 2 + r
                    rg = m % 4
                    nc.tensor.matmul(
                        t_ps[:, r * IW : (r + 1) * IW],
                        v_t[32 * rg : 32 * rg + 4, g * 128 : (g + 1) * 128],
                        u_t[32 * rg : 32 * rg + 4, g * IW : (g + 1) * IW],
                        start=True,
                        stop=True,
                        tile_position=(32 * rg, 0),
                    )
                q_t = qpool.tile([128, 2 * IW], BF16)
                nc.scalar.square(q_t[:], t_ps[:])
                w_t = wpool.tile([128, 2 * IW], BF16)
                nc.vector.scalar_tensor_tensor(
                    w_t[:], t_ps[:], 0.0, q_t[:],
                    op0=mybir.AluOpType.max, op1=mybir.AluOpType.mult,
                )
                for r in range(2):
                    pend.append((g * 4 + half * 2 + r, w_t, r))
            while len(pend) > 4 or (g == ngrp - 1 and pend):
                m, w_t, r = pend.pop(0)
                par = (m % 2) * C_OUT
                nc.tensor.matmul(
                    out_ps[par : par + C_OUT, :],
                    at_t[:, m * C_OUT : (m + 1) * C_OUT],
                    w_t[:, r * IW : (r + 1) * IW],
                    start=(m < 2),
                    stop=(m >= m_items - 2),
                    skip_group_check=True,
                    tile_position=(0, par),
                )

        tmp_sb = const.tile([C_OUT, IW], F32)
        nc.scalar.copy(tmp_sb[:], out_ps[0:C_OUT, :])
        out_sb = const.tile([C_OUT, IW], F32)
        nc.vector.tensor_add(out_sb[:], tmp_sb[:], out_ps[C_OUT:, :])
        nc.sync.dma_start(out_d[:], out_sb[:])
    _split_sync_waits(nc)
    return nc


def _get_nc(m_items):
    key = ("nc", m_items)
    if key not in _cache:
        _cache[key] = _build_nc(m_items)
    return _cache[key]


# ------------------------------------------------------------ host wrapper
def _offsets():
    axes = [
        (np.arange(kk) - (kk - 1) / 2.0) * d for kk, d in zip(KSIZE, DILATION)
    ]
    grids = np.meshgrid(*axes, indexing="ij")
    return np.stack([g.reshape(-1) for g in grids], axis=-1).astype(np.float32)


def _kd_sort(p):
    """Sort by x: i-blocks and j-chunks become equal-count vertical strips.
    (Empirically the best-balanced partition for this workload: M=72.)"""
    return np.argsort(p[:, 0], kind="stable")


def _prepare_in_maps(locs, data, density, weight, bias):
    locs = np.asarray(locs, np.float32)
    data = np.asarray(data, np.float32)
    density = np.asarray(density, np.float32)
    weight = np.asarray(weight, np.float32)

    pos = locs[..., :NDIM]                       # [B,N,2]
    invmass = locs[..., NDIM]                    # [B,N]
    coef = 1.0 / (invmass * density)             # [B,N]
    dcoef = data * coef[:, None, :]              # [B,C,N]
    offs = _offsets()                            # [9,2]

    n_iblk = N // IW
    n_jchunk = N // 128

    perms = []
    tiles_per_block = []   # (b, iq) -> list[(k, chunk)]
    spos_all = []
    for b in range(B):
        perm = _kd_sort(pos[b])
        perms.append(perm)
        sp = pos[b][perm]
        spos_all.append(sp)
        jlo = sp.reshape(n_jchunk, 128, 2).min(1)
        jhi = sp.reshape(n_jchunk, 128, 2).max(1)
        ilo = sp.reshape(n_iblk, IW, 2).min(1)
        ihi = sp.reshape(n_iblk, IW, 2).max(1)
        for q in range(n_iblk):
            items = []
            for k in range(NCELLS):
                lo_i = ilo[q] + offs[k]
                hi_i = ihi[q] + offs[k]
                dx = np.maximum(0, np.maximum(jlo[:, 0] - hi_i[0],
                                              lo_i[0] - jhi[:, 0]))
                dy = np.maximum(0, np.maximum(jlo[:, 1] - hi_i[1],
                                              lo_i[1] - jhi[:, 1]))
                for c in np.nonzero(dx * dx + dy * dy < R2)[0]:
                    items.append((k, int(c)))
            tiles_per_block.append(items)

    m_items = max(len(t) for t in tiles_per_block)
    m_items = -(-m_items // 8) * 8  # round up to multiple of 8

    in_maps = []
    for b in range(B):
        perm = perms[b]
        sp = spos_all[b]
        x, y = sp[:, 0], sp[:, 1]
        n2 = x * x + y * y
        # sorted-order A_kT[j, o] (knorm folded): [9, N, C_OUT]
        a_t = (np.einsum("ock,cj->kjo", weight, dcoef[b][:, perm]) * KNORM
               ).astype(ml_dtypes.bfloat16)
        # per-k V rows over sorted j, U rows over sorted i
        v_h = np.empty((NCELLS, 4, N), np.float32)
        u_h = np.empty((NCELLS, 4, N), np.float32)
        for k in range(NCELLS):
            ox, oy = float(offs[k, 0]), float(offs[k, 1])
            v_h[k, 0] = 2.0 * x
            v_h[k, 1] = 2.0 * y
            v_h[k, 2] = 1.0
            v_h[k, 3] = -n2 + 2.0 * (ox * x + oy * y)
            u_h[k, 0] = x
            u_h[k, 1] = y
            u_h[k, 2] = R2 - (ox * ox + oy * oy) - n2 - 2.0 * (ox * x + oy * y)
            u_h[k, 3] = 1.0

        ngrp = m_items // 4
        for q in range(n_iblk):
            items = tiles_per_block[b * n_iblk + q]
            at_core = np.zeros((128, m_items * C_OUT), ml_dtypes.bfloat16)
            v_core = np.zeros((128, ngrp * 128), np.float32)
            u_core = np.zeros((128, ngrp * IW), np.float32)
            for m, (k, c) in enumerate(items):
                g, r = divmod(m, 4)
                at_core[:, m * C_OUT : (m + 1) * C_OUT] = \
                    a_t[k, c * 128 : (c + 1) * 128]
                v_core[32 * r : 32 * r + 4, g * 128 : (g + 1) * 128] = \
                    v_h[k][:, c * 128 : (c + 1) * 128]
                u_core[32 * r : 32 * r + 4, g * IW : (g + 1) * IW] = \
                    u_h[k][:, q * IW : (q + 1) * IW]
            in_maps.append({"AT": at_core, "V": v_core, "U": u_core})
    return in_maps, m_items, perms


def _run(in_maps, m_items):
    nc = _get_nc(m_items)
    return run_bass_kernel_spmd(nc, in_maps, list(range(NCORES)))


def kernel(locs, data, density, weight, bias):
    in_maps, m_items, perms = _prepare_in_maps(locs, data, density, weight, bias)
    res = _run(in_maps, m_items)
    bias = np.asarray(bias, np.float32)
    n_iblk = N // IW
    out = np.empty((B, C_OUT, N), np.float32)
    for b in range(B):
        sorted_out = np.concatenate(
            [res.results[b * n_iblk + q]["out"] for q in range(n_iblk)], axis=1
        )
        out[b][:, perms[b]] = sorted_out + bias[:, None]
    return out


# -------------------------------------------------------------- benchmarking
def time_kernel(locs, data, density, weight, bias, iters=12):
    """Return (best_wall_s, per_call_s_list) for the device launch only."""
    in_maps, m_items, _ = _prepare_in_maps(locs, data, density, weight, bias)
    _run(in_maps, m_items)  # warm (compile)
    times = []
    for _ in range(iters):
        t0 = time.perf_counter()
        _run(in_maps, m_items)
        times.append(time.perf_counter() - t0)
    return min(times), times



# revision 11
# speedup vs baseline: 9321.7358x; 9321.7358x over previous
"""ConvSP (SPH smoothing-kernel convolution) Trainium2 Bass kernel.

Math (per batch b):
  out[o,i] = bias[o] + sum_k sum_j A_k[o,j] * relu(r^2 - |x_i - x_j + off_k|^2)^3
  A_k = knorm * weight[:,:,k] @ (data * coef),  coef = 1/(invmass*density)

Device strategy (8 cores = 2 batches x 4 i-blocks of 512):
  t_k[j,i] = r2 - |x_i - x_j + off_k|^2 is a rank-4 bilinear form:
      V_k[:,j] = [2x_j, 2y_j, 1, -|x_j|^2 + 2 x_j.off_k]          (lhsT, K=4)
      U_k[:,i] = [x_i, y_i, r2 - |off_k|^2 - |x_i|^2 - 2 x_i.off_k, 1]
  so each [128j x 512i] tile of t is ONE fp32 matmul. Then a single custom
  DVE op computes w = relu(t)^3 (PSUM fp32 -> SBUF bf16), and a bf16 matmul
  accumulates out[o,i] += A_kT[j,o].T @ w[j,i] into PSUM over all (k, j-chunk).
"""

import os
import re
import sys
import time

import numpy as np

for _p in ("/opt/trn_rl_repo", "/root/.axon_site/_ro/trn_rl_repo"):
    if os.path.isdir(_p) and _p not in sys.path:
        sys.path.append(_p)

import ml_dtypes  # noqa: E402

import concourse.bass as bass  # noqa: E402
import concourse.mybir as mybir  # noqa: E402
import concourse.tile as tile  # noqa: E402
from concourse.bass_utils import run_bass_kernel_spmd  # noqa: E402

# ---------------------------------------------------------------- constants
NDIM = 2
KSIZE = (3, 3)
DILATION = (0.05, 0.05)
RADIUS = 0.1
C_IN = 64
C_OUT = 64
B = 2
N = 2048
NCELLS = 9
R2 = RADIUS * RADIUS
KNORM = 315.0 / (64.0 * np.pi * RADIUS**9)

NCORES = 8
IBLK = 512          # i-columns per core
CHUNKS = N // 128   # 16 j-chunks of 128
KR = 10             # bf16 hi/lo-split rows per work item (was 4 fp32 rows)

F32 = mybir.dt.float32
BF16 = mybir.dt.bfloat16

_cache: dict = {}


# ------------------------------------------------- TileContext drain patch
# The walrus in this container rejects the Tile tail-drain when it carries
# more than ~2 sem waits ("Too many sync wait commands"). Split the waits
# over extra sync-engine NOPs, one wait each.
def _patch_tile_drain():
    if getattr(tile.TileContext, "_drain_patched", False):
        return
    import bass_rust
    from concourse.vector_clock import ScopedClock

    def _drain_and_barrier(self, tick_clock, wait_clock):
        drain_inst = self.nc.sync.drain()
        wait_clock.add_sem_waits(
            drain_inst.ins, ScopedClock({None: tick_clock.global_clock})
        )
        si = drain_inst.ins.sync_info
        waits = list(si.on_wait) if si is not None else []
        if len(waits) > 1:
            si.on_wait = waits[:1]
            drain_inst.ins.sync_info = si
            for w in waits[1:]:
                n = self.nc.sync.nop(nofuse=True, hint="drain_wait_split")
                n.ins.sync_info = bass_rust.SyncInfo(on_wait=[w], on_update=[])
        self.nc.all_engine_barrier()
        popped = self.nc._tile_sem_poison_stack.pop()
        assert popped is self._sem_poison
        self.nc.clear_and_free_semaphores(list(self.sems.allocated().values()))
        self.nc.all_engine_barrier()

    tile.TileContext._drain_and_barrier = _drain_and_barrier
    tile.TileContext._drain_patched = True


# --------------------------------------------- sync-wait legalization pass
# This walrus rejects instructions carrying more than ~1-2 sem waits. After
# Tile scheduling, move excess waits onto same-engine NoOps inserted right
# before the over-subscribed instruction (engines execute their stream in
# order, so semantics are identical).
_WAIT_LIMIT = 1


def _split_sync_waits(nc, limit=_WAIT_LIMIT):
    cnt = 0
    for f in nc.m.functions:
        for bb in f.blocks:
            changed = False
            out = []
            for inst in bb.instructions:
                si = inst.sync_info
                waits = list(si.on_wait) if si is not None else []
                if len(waits) > limit:
                    keep = waits[-limit:]
                    excess = waits[:-limit]
                    for j in range(0, len(excess), limit):
                        n = mybir.InstNoOp(
                            name=f"waitsplit_{cnt}",
                            engine=inst.engine,
                            ins=[],
                            outs=[],
                            sync_info=mybir.SyncInfo(
                                on_wait=excess[j : j + limit], on_update=[]
                            ),
                        )
                        cnt += 1
                        nc.register_instruction(n, overwrite=True)
                        out.append(n)
                    si.on_wait = keep
                    inst.sync_info = si
                    changed = True
                out.append(inst)
            if changed:
                bb.instructions = out
    return cnt


# ------------------------------------------------- custom DVE op: relu(x)^3
def _get_relu_cube():
    """Register (once) and return the RELU_CUBE custom DVE op, or None."""
    if not os.environ.get("KERNEL_USE_CUSTOM_DVE"):
        return None
    if "relu_cube" in _cache:
        return _cache["relu_cube"]
    try:
        import concourse.dve_ops as dve_ops
        from concourse.dve_ops import DveOp
        from concourse.dve_spec import Spec, Src0, relu, sq

        name = "RELU_CUBE_ANT"
        r = relu(Src0)
        spec = Spec(
            body=sq(r) * r,
            reference=lambda in0, in1, s0, s1, imm2: (
                np.maximum(in0, 0.0) ** 3
            ).astype(np.float32),
        )
        if name not in dve_ops._SUB_OPCODE_FOR_NAME:
            placeholder = DveOp(name, spec, subdim=False, uops_sha={})
            dve_ops.OPS.append(placeholder)
            dve_ops._SUB_OPCODE_FOR_NAME[name] = (
                dve_ops._CUSTOM_DVE_ROW_BASE + len(dve_ops.OPS) - 1
            )
            assert dve_ops._SUB_OPCODE_FOR_NAME[name] < 0x20
            dve_ops.CUSTOM_DVE_SPECS[name] = spec
        # pin the uops sha by compiling once and parsing the mismatch error
        shas = {}
        for ver in ("v3", "v4"):
            try:
                dve_ops.OPS[-1].compile(ver)
            except ValueError as e:
                m = re.search(r"\b([0-9a-f]{8,})\b\s*≠", str(e))
                if m:
                    shas[ver] = m.group(1)
            except Exception:
                pass
        op = DveOp(name, spec, subdim=False, uops_sha=shas)
        dve_ops.OPS[-1] = op
        dve_ops.CUSTOM_DVE_SPECS[name] = spec
        # verify it now compiles clean for v3 (trn2)
        op.compile("v3")
        _cache["relu_cube"] = op
    except Exception as e:  # pragma: no cover - fallback path
        sys.stderr.write(f"[kernel] custom DVE unavailable ({e}); fallback\n")
        _cache["relu_cube"] = None
    return _cache["relu_cube"]


# ------------------------------------------------------------- device build
# Sharding: core = (b, i-block of 512 kd-sorted particles). Block-sparse:
# the host kd-sorts particles, tests (k, j-chunk) tiles against the core's
# i-block bounding box, and ships only surviving tiles as "work items" in
# item-indexed V/U/AT arrays (padded with zeros up to a common M).
IW = 512             # i-columns per core


def _build_nc(m_items):
    _patch_tile_drain()
    relu_cube = _get_relu_cube()
    # tiles routed to ACT(square)+GPSIMD(stt) instead of the single-pass
    # DVE relu-cube: every 'APG'th tile (0 disables the gpsimd path)
    apg = int(os.environ.get("KERNEL_APG", "0"))

    ngrp = m_items // 4
    nc = bass.Bass()
    # V/U packed over 4 row-groups: item m lives at partitions 32*(m%4)..+KR,
    # column block m//4 (4-way concurrent row-tiled t-matmuls). Each fp32
    # row is hi/lo bf16-split (KR=10 rows/item, see _prepare_in_maps), so
    # the t-matmul runs at full bf16 rate with ~1e-5 absolute error.
    # DRAM V/U are packed dense ([4*KR rows]); one DMA per row band.
    at_d = nc.declare_dram_parameter("AT", [128, m_items * C_OUT], BF16,
                                     isOutput=False)
    v_d = nc.declare_dram_parameter("V", [4 * KR, ngrp * 128], BF16,
                                    isOutput=False)
    u_d = nc.declare_dram_parameter("U", [4 * KR, ngrp * IW], BF16,
                                    isOutput=False)
    out_d = nc.declare_dram_parameter("out", [C_OUT, IW], F32, isOutput=True)

    from contextlib import ExitStack

    with tile.TileContext(nc) as tc, ExitStack() as ctx:
        const = ctx.enter_context(tc.tile_pool(name="const", bufs=1))
        wpool = ctx.enter_context(tc.tile_pool(name="w", bufs=4))
        qpool = ctx.enter_context(tc.tile_pool(name="q", bufs=4))
        tpool = ctx.enter_context(tc.tile_pool(name="t", bufs=3, space="PSUM"))
        opool = ctx.enter_context(tc.tile_pool(name="o", bufs=1, space="PSUM"))

        v_t = const.tile([128, ngrp * 128], BF16)
        u_t = const.tile([128, ngrp * IW], BF16)
        for rg in range(4):
            nc.sync.dma_start(v_t[32 * rg : 32 * rg + KR, :],
                              v_d[KR * rg : KR * rg + KR, :])
            half_u = ngrp * IW // 2
            for qq in range(2):
                nc.sync.dma_start(
                    u_t[32 * rg : 32 * rg + KR,
                        qq * half_u : (qq + 1) * half_u],
                    u_d[KR * rg : KR * rg + KR,
                        qq * half_u : (qq + 1) * half_u])
        at_t = const.tile([128, m_items * C_OUT], BF16)
        qsz = m_items * C_OUT // 4
        for qq in range(4):
            nc.sync.dma_start(at_t[:, qq * qsz : (qq + 1) * qsz],
                              at_d[:, qq * qsz : (qq + 1) * qsz])

        # main-matmul accumulator: even items -> partitions 0:64,
        # odd items -> 64:128 (2-way col-tiled concurrency); added at the end
        out_ps = opool.tile([128, IW], F32)

        # software pipeline: t-matmuls for group g run while DVE/ACT cube
        # group g-1 and PE then accumulates g-1's mains. Items are paired
        # into [128, 2*IW] PSUM tiles so each ACT/DVE op covers two items.
        pend = []  # (item_idx, w_tile, half) awaiting the main matmul
        tile_no = 0
        for g in range(ngrp):
            for half in range(2):
                t_ps = tpool.tile([128, 2 * IW], F32)
                for r in range(2):
                    m = g * 4 + half * 2 + r
                    rg = m % 4
                    nc.tensor.matmul(
                        t_ps[:, r * IW : (r + 1) * IW],
                        v_t[32 * rg : 32 * rg + KR,
                            g * 128 : (g + 1) * 128],
                        u_t[32 * rg : 32 * rg + KR,
                            g * IW : (g + 1) * IW],
                        start=True,
                        stop=True,
                        tile_position=(32 * rg, 0),
                    )
                w_t = wpool.tile([128, 2 * IW], BF16)
                tile_no += 1
                if relu_cube is None:
                    q_t = qpool.tile([128, 2 * IW], BF16)
                    nc.scalar.square(q_t[:], t_ps[:])
                    nc.vector.scalar_tensor_tensor(
                        w_t[:], t_ps[:], 0.0, q_t[:],
                        op0=mybir.AluOpType.max, op1=mybir.AluOpType.mult,
                    )
                elif apg and tile_no % apg == 0:
                    # PSUM-free side route: ACT relu + ACT square,
                    # GPSIMD multiplies the SBUF pair (can't read PSUM).
                    c_t = qpool.tile([128, 2 * IW], BF16)
                    nc.scalar.activation(
                        c_t[:], t_ps[:],
                        mybir.ActivationFunctionType.Relu, 0.0, 1.0)
                    q_t = qpool.tile([128, 2 * IW], BF16)
                    nc.scalar.square(q_t[:], c_t[:])
                    nc.gpsimd.tensor_mul(w_t[:], q_t[:], c_t[:])
                else:
                    nc.vector._custom_dve(relu_cube, out=w_t[:], in0=t_ps[:])
                for r in range(2):
                    pend.append((g * 4 + half * 2 + r, w_t, r))
            while len(pend) > 4 or (g == ngrp - 1 and pend):
                m, w_t, r = pend.pop(0)
                par = (m % 2) * C_OUT
                nc.tensor.matmul(
                    out_ps[par : par + C_OUT, :],
                    at_t[:, m * C_OUT : (m + 1) * C_OUT],
                    w_t[:, r * IW : (r + 1) * IW],
                    start=(m < 2),
                    stop=(m >= m_items - 2),
                    skip_group_check=True,
                    tile_position=(0, par),
                )

        tmp_sb = const.tile([C_OUT, IW], F32)
        nc.scalar.copy(tmp_sb[:], out_ps[0:C_OUT, :])
        out_sb = const.tile([C_OUT, IW], F32)
        nc.vector.tensor_add(out_sb[:], tmp_sb[:], out_ps[C_OUT:, :])
        nc.sync.dma_start(out_d[:], out_sb[:])
    _split_sync_waits(nc)
    return nc


def _get_nc(m_items):
    key = ("nc", m_items)
    if key not in _cache:
        _cache[key] = _build_nc(m_items)
    return _cache[key]


# ------------------------------------------------------------ host wrapper
def _offsets():
    axes = [
        (np.arange(kk) - (kk - 1) / 2.0) * d for kk, d in zip(KSIZE, DILATION)
    ]
    grids = np.meshgrid(*axes, indexing="ij")
    return np.stack([g.reshape(-1) for g in grids], axis=-1).astype(np.float32)


def _kd_sort(p):
    """Sort by x: i-blocks and j-chunks become equal-count vertical strips.
    (Empirically the best-balanced partition for this workload: M=72.)"""
    return np.argsort(p[:, 0], kind="stable")


def _prepare_in_maps(locs, data, density, weight, bias):
    locs = np.asarray(locs, np.float32)
    data = np.asarray(data, np.float32)
    density = np.asarray(density, np.float32)
    weight = np.asarray(weight, np.float32)

    pos = locs[..., :NDIM]                       # [B,N,2]
    invmass = locs[..., NDIM]                    # [B,N]
    coef = 1.0 / (invmass * density)             # [B,N]
    dcoef = data * coef[:, None, :]              # [B,C,N]
    offs = _offsets()                            # [9,2]

    n_iblk = N // IW
    n_jchunk = N // 128

    perms = []
    tiles_per_block = []   # (b, iq) -> list[(k, chunk)]
    spos_all = []
    for b in range(B):
        perm = _kd_sort(pos[b])
        perms.append(perm)
        sp = pos[b][perm]
        spos_all.append(sp)
        jlo = sp.reshape(n_jchunk, 128, 2).min(1)
        jhi = sp.reshape(n_jchunk, 128, 2).max(1)
        ilo = sp.reshape(n_iblk, IW, 2).min(1)
        ihi = sp.reshape(n_iblk, IW, 2).max(1)
        for q in range(n_iblk):
            items = []
            for k in range(NCELLS):
                lo_i = ilo[q] + offs[k]
                hi_i = ihi[q] + offs[k]
                dx = np.maximum(0, np.maximum(jlo[:, 0] - hi_i[0],
                                              lo_i[0] - jhi[:, 0]))
                dy = np.maximum(0, np.maximum(jlo[:, 1] - hi_i[1],
                                              lo_i[1] - jhi[:, 1]))
                for c in np.nonzero(dx * dx + dy * dy < R2)[0]:
                    items.append((k, int(c)))
            tiles_per_block.append(items)

    m_items = max(len(t) for t in tiles_per_block)
    m_items = -(-m_items // 8) * 8  # round up to multiple of 8

    in_maps = []
    for b in range(B):
        perm = perms[b]
        sp = spos_all[b]
        x, y = sp[:, 0], sp[:, 1]
        n2 = x * x + y * y
        # sorted-order A_kT[j, o] (knorm folded): [9, N, C_OUT]
        a_t = (np.einsum("ock,cj->kjo", weight, dcoef[b][:, perm]) * KNORM
               ).astype(ml_dtypes.bfloat16)

        # hi/lo bf16 split: v = hi + lo exactly representable products on PE
        bf = ml_dtypes.bfloat16

        def _hl(a):
            h = a.astype(bf)
            return h, (a - h.astype(np.float32)).astype(bf)

        # per-k V rows over sorted j, U rows over sorted i; KR=10 pairing:
        #   t = 2x_j*x_i + 2y_j*y_i + s_i + q_j
        #   p0..2: hi(2x)*hi(xi), hi(2x)*lo(xi), lo(2x)*hi(xi)
        #   p3..5: same for y; p6,7: 1*hi(s), 1*lo(s); p8,9: hi(q)*1, lo(q)*1
        v_h = np.empty((NCELLS, KR, N), bf)
        u_h = np.empty((NCELLS, KR, N), bf)
        x2h, x2l = _hl(2.0 * x)
        y2h, y2l = _hl(2.0 * y)
        xh, xl = _hl(x)
        yh, yl = _hl(y)
        for k in range(NCELLS):
            ox, oy = float(offs[k, 0]), float(offs[k, 1])
            qv = -n2 + 2.0 * (ox * x + oy * y)
            sv = R2 - (ox * ox + oy * oy) - n2 - 2.0 * (ox * x + oy * y)
            qh, ql = _hl(qv)
            sh, sl = _hl(sv)
            one = np.ones_like(x, bf)
            v_h[k] = np.stack([x2h, x2h, x2l, y2h, y2h, y2l,
                               one, one, qh, ql])
            u_h[k] = np.stack([xh, xl, xh, yh, yl, yh, sh, sl, one, one])

        ngrp = m_items // 4
        for q in range(n_iblk):
            items = tiles_per_block[b * n_iblk + q]
            at_core = np.zeros((128, m_items * C_OUT), bf)
            v_core = np.zeros((4 * KR, ngrp * 128), bf)
            u_core = np.zeros((4 * KR, ngrp * IW), bf)
            for m, (k, c) in enumerate(items):
                g, r = divmod(m, 4)
                at_core[:, m * C_OUT : (m + 1) * C_OUT] = \
                    a_t[k, c * 128 : (c + 1) * 128]
                v_core[KR * r : KR * r + KR, g * 128 : (g + 1) * 128] = \
                    v_h[k][:, c * 128 : (c + 1) * 128]
                u_core[KR * r : KR * r + KR, g * IW : (g + 1) * IW] = \
                    u_h[k][:, q * IW : (q + 1) * IW]
            in_maps.append({"AT": at_core, "V": v_core, "U": u_core})
    return in_maps, m_items, perms


def _run(in_maps, m_items):
    nc = _get_nc(m_items)
    return run_bass_kernel_spmd(nc, in_maps, list(range(NCORES)))


def kernel(locs, data, density, weight, bias):
    in_maps, m_items, perms = _prepare_in_maps(locs, data, density, weight, bias)
    res = _run(in_maps, m_items)
    bias = np.asarray(bias, np.float32)
    n_iblk = N // IW
    out = np.empty((B, C_OUT, N), np.float32)
    for b in range(B):
        sorted_out = np.concatenate(
            [res.results[b * n_iblk + q]["out"] for q in range(n_iblk)], axis=1
        )
        out[b][:, perms[b]] = sorted_out + bias[:, None]
    return out


# -------------------------------------------------------------- benchmarking
def time_kernel(locs, data, density, weight, bias, iters=12):
    """Return (best_wall_s, per_call_s_list) for the device launch only."""
    in_maps, m_items, _ = _prepare_in_maps(locs, data, density, weight, bias)
    _run(in_maps, m_items)  # warm (compile)
    times = []
    for _ in range(iters):
        t0 = time.perf_counter()
        _run(in_maps, m_items)
        times.append(time.perf_counter() - t0)
    return min(times), times

